# revision 14
# baseline (speedup 1.0000x reference)
"""Trainium2 Bass kernel for nn_AdaptiveModalityEncoder.

Reference computation (per row r of input_data [B, D]):
    sel[r] = selection_mask[r, modality_idx] > 0.5
    out[r] = sel[r] ? gelu(x[r] @ W1 + b1) @ W2 + b2 : 0

Strategy (moe_routing, data-parallel across 8 cores):
  - Host computes the selected-row list, gathers + transposes the selected
    rows (routing metadata/prep), and splits them evenly across the 8
    cores; each core runs a pure dense 2-layer MLP in bf16 (fp32
    accumulate) over its rows and writes a compact batch-major bf16
    output. Host scatters the compact outputs into the zero-filled full
    output.
  - Device kernel is gather/scatter-free: only linear DMAs. Inputs are
    packed host-side into few wide-line DMAs (8-32 KB per partition
    line) because DMA descriptor generation (~9 ns/line, 128 lines per
    SBUF tile) otherwise limits the startup; biases ride along inside
    the X^T and W2 transfers.
  - The Tensor engine p-state ramps over ~5 us of continuous execution
    (427 -> 216 ns per 512-col matmul) and resets on multi-us idle, so
    a short burst of garbage warm-up matmuls covers the engine-boot +
    first-DMA window and real work starts at full speed with zero
    stalls.

Matmul layout: L1 feature-major (H^T = W1^T @ X^T; X^T pre-transposed on
host, W1 repacked so each h-tile's k-slices are contiguous); L2 flips
operands (OUT = (HT)^T @ W2, W2 repacked k-major) so output rows come out
batch-major and stream straight to DRAM.
"""

import sys

sys.path.insert(0, "/opt/trn_rl_repo")

import numpy as np
import ml_dtypes

# Problem constants (hardcoded per harness contract).
B, D, H, O, K = 16384, 1024, 2048, 1024, 4
NCORES = 8
P = 128
KD = D // P  # 8 k-tiles for layer 1
KH = H // P  # 16 k-tiles for layer 2

# W1 h-tile DMA grouping: singles first (feed the first chains ASAP),
# then quads (fewer descriptors per byte).
W1_GROUPS = [[0], [1], [2, 3, 4, 5], [6, 7, 8, 9], [10, 11, 12, 13], [14, 15]]

_GRAPH_CACHE = {}


def _chunks_of(CP):
    out = []
    c0 = 0
    while c0 < CP:
        w = min(512, CP - c0)
        out.append((c0, w))
        c0 += w
    return out


def _build_graph(NG, has_b2, act="gelu", NWU=8):
    """Build + compile the per-core Bass graph. NG = number of 128-row
    tiles per core (CP = NG*128 padded rows). Same graph on all 8 cores."""
    import concourse.mybir as mybir
    import concourse.tile as tile
    from concourse import bacc

    f32 = mybir.dt.float32
    bf16 = mybir.dt.bfloat16
    act_fn = {
        "gelu": mybir.ActivationFunctionType.Gelu_apprx_tanh,
        "tanh": mybir.ActivationFunctionType.Tanh,  # CoreSim stand-in
    }[act]
    copy_fn = mybir.ActivationFunctionType.Copy

    CP = NG * P  # padded rows per core
    chunks = _chunks_of(CP)
    NC = len(chunks)

    nc = bacc.Bacc("TRN2", target_bir_lowering=False, debug=False, num_devices=NCORES)

    # X^T packed per chunk (each chunk's KD k-tiles contiguous), with the
    # b1 bias columns folded in after chunk 0.
    xt_cols = KD * CP + KH
    xt_d = nc.dram_tensor("xt", [P, xt_cols], bf16, kind="ExternalInput")
    w1r_d = nc.dram_tensor("w1r", [P, KH * KD * P], bf16, kind="ExternalInput")
    w2_cols = KH * O + (O if has_b2 else 0)
    w2r_d = nc.dram_tensor("w2r", [P, w2_cols], bf16, kind="ExternalInput")
    out_d = nc.dram_tensor("out", [CP, O], bf16, kind="ExternalOutput")

    with tile.TileContext(nc) as tc:
        with (
            tc.tile_pool(name="w1pool", bufs=len(W1_GROUPS)) as w1pool,
            tc.tile_pool(name="w2pool", bufs=4) as w2pool,
            tc.tile_pool(name="xtp", bufs=NC) as xtp,
            tc.tile_pool(name="htp", bufs=KH) as htp,
            tc.tile_pool(name="outp", bufs=4) as outp,
            tc.tile_pool(name="const", bufs=1) as constp,
            tc.tile_pool(name="ps1", bufs=3, space="PSUM") as ps1,  # layer 1
            tc.tile_pool(name="ps2", bufs=3, space="PSUM") as ps2,  # layer 2
            tc.tile_pool(name="wup", bufs=1, space="PSUM") as wup,
        ):
            # ---- PE warm-up (see module docstring) ----
            wu = constp.tile([P, 512], bf16)
            nc.gpsimd.memset(wu[:], 0.0)
            # DMA-engine warm-up: the physical DMA engines serve their first
            # ~2 us of descriptors at half speed. gpsimd comes alive ~2 us
            # before the sync sequencer's stream, so a throwaway SWDGE load
            # issued here absorbs the ramp and the real input stream below
            # runs at full rate from its first descriptor.
            wdma = constp.tile([P, 2048], bf16)
            nc.gpsimd.dma_start(wdma[:], w1r_d[:, 0:2048])
            wuacc = wup.tile([P, 512], f32)
            for i in range(NWU):
                nc.tensor.matmul(
                    wuacc[:], wu[:, 0:P], wu[:], start=True, stop=True
                )

            # ---- DMA issue order = generation order: critical-path first
            # chunk 0 of X^T (+ b1 columns) as ONE wide DMA
            cw0 = chunks[0][1]
            xt0_sb = xtp.tile([P, KD * cw0 + KH], bf16, tag="xt0", name="xt0")
            nc.sync.dma_start(xt0_sb[:], xt_d[:, 0 : KD * cw0 + KH])
            b1_ap = xt0_sb[:, KD * cw0 : KD * cw0 + KH]

            # W1 h-tiles: two singles, then grouped
            w1_tiles = []
            for gi, grp in enumerate(W1_GROUPS):
                t = w1pool.tile(
                    [P, len(grp) * KD * P], bf16, tag="w1", name=f"w1g{gi}"
                )
                w1_tiles.append(t)
                h0 = grp[0]
                nc.sync.dma_start(
                    t[:], w1r_d[:, h0 * KD * P : (h0 + len(grp)) * KD * P]
                )
            # h -> (tile, col offset of that h's KD*P block)
            w1_slice = {}
            for gi, grp in enumerate(W1_GROUPS):
                for j, h in enumerate(grp):
                    w1_slice[h] = (w1_tiles[gi], j * KD * P)

            # remaining X^T chunks
            xt_sb = [None] * NC
            off = KD * cw0 + KH
            for ci in range(1, NC):
                cw = chunks[ci][1]
                t = xtp.tile([P, KD * cw], bf16, tag="xt", name=f"xt{ci}")
                xt_sb[ci] = t
                nc.sync.dma_start(t[:], xt_d[:, off : off + KD * cw])
                off += KD * cw

            # W2 in 4 quads of k-tiles; b2 columns ride in the last quad
            w2_tiles = []
            for qi in range(4):
                extra = O if (has_b2 and qi == 3) else 0
                t = w2pool.tile([P, 4 * O + extra], bf16, tag="w2", name=f"w2q{qi}")
                w2_tiles.append(t)
                nc.sync.dma_start(
                    t[:], w2r_d[:, qi * 4 * O : (qi + 1) * 4 * O + extra]
                )
            b2_ap = w2_tiles[3][:, 4 * O : 5 * O] if has_b2 else None

            def xt_k(ci, k):
                cw = chunks[ci][1]
                t = xt0_sb if ci == 0 else xt_sb[ci]
                return t[:, k * cw : (k + 1) * cw]

            def w2_k(k, lo, hi):
                t = w2_tiles[k // 4]
                base = (k % 4) * O
                return t[:, base + lo : base + hi]

            ht_sb = [
                htp.tile([P, CP], bf16, tag="ht", name=f"htsb{h}")
                for h in range(KH)
            ]

            # ---- compute: per column chunk, L1 then L2 ----
            for ci, (c0, cw) in enumerate(chunks):
                # layer 1: H^T chunk = gelu(W1^T @ X^T + b1)
                for h in range(KH):
                    acc = ps1.tile([P, cw], f32, tag="l1acc", name=f"l1a{ci}_{h}")
                    w1t, w1off = w1_slice[h]
                    for k in range(KD):
                        nc.tensor.matmul(
                            acc[:],
                            w1t[:, w1off + k * P : w1off + (k + 1) * P],
                            xt_k(ci, k),
                            start=(k == 0),
                            stop=(k == KD - 1),
                        )
                    nc.scalar.activation(
                        ht_sb[h][:, c0 : c0 + cw],
                        acc[:],
                        act_fn,
                        bias=b1_ap[:, h : h + 1],
                    )

                # layer 2, batch-major: OUT rows = (HT slice)^T @ W2 + b2
                for rl in range(cw // P):
                    r0 = c0 + rl * P
                    for oc in range(2):
                        acc2 = ps2.tile(
                            [P, 512], f32, tag="l2acc", name=f"l2a{ci}_{rl}_{oc}"
                        )
                        for k in range(KH):
                            nc.tensor.matmul(
                                acc2[:],
                                ht_sb[k][:, r0 : r0 + P],
                                w2_k(k, oc * 512, (oc + 1) * 512),
                                start=(k == 0),
                                stop=(k == KH - 1),
                            )
                        ob = outp.tile(
                            [P, 512], bf16, tag="outsb", name=f"osb{ci}_{rl}_{oc}"
                        )
                        # Evict + write on the scalar engine only: its HWDGE
                        # queue fires ~30 ns after the eviction (same-engine
                        # dependency), where a cross-engine hop costs ~0.7 us.
                        if has_b2:
                            nc.vector.tensor_add(
                                ob[:], acc2[:], b2_ap[:, oc * 512 : (oc + 1) * 512]
                            )
                        else:
                            nc.scalar.activation(ob[:], acc2[:], copy_fn)
                        nc.scalar.dma_start(
                            out_d[r0 : r0 + P, oc * 512 : (oc + 1) * 512], ob[:]
                        )

    nc.compile()
    return nc


def _get_graph(NG, has_b2, act="gelu", NWU=8):
    key = (NG, has_b2, act, NWU)
    if key not in _GRAPH_CACHE:
        _GRAPH_CACHE[key] = _build_graph(NG, has_b2, act, NWU)
    return _GRAPH_CACHE[key]


def prepare(input_data, selection_mask, W1, b1, W2, b2, modality_idx, act="gelu", NWU=8):
    """Host-side routing/sharding prep. Returns (nc, in_maps, meta) or None
    if no rows are selected (output is all zeros)."""
    x = np.asarray(input_data, dtype=np.float32)
    mask = np.asarray(selection_mask, dtype=np.float32)
    midx = int(np.asarray(modality_idx))
    rows = np.nonzero(mask[:, midx] > 0.5)[0]
    total = len(rows)
    if total == 0:
        return None

    T = -(-total // NCORES)  # rows per core
    NG = -(-T // P)
    CP = NG * P
    chunks = _chunks_of(CP)
    has_b2 = bool(np.any(np.asarray(b2)))

    nc = _get_graph(NG, has_b2, act, NWU)

    bf = ml_dtypes.bfloat16
    x_bf = x.astype(bf)
    # W1 repacked: block (h, k) at cols (h*KD + k)*P
    w1r = np.ascontiguousarray(
        np.asarray(W1, dtype=np.float32)
        .astype(bf)
        .reshape(KD, P, KH, P)
        .transpose(1, 2, 0, 3)
        .reshape(P, KH * KD * P)
    )
    # W2 repacked k-major: block k at cols k*O
    w2r = (
        np.asarray(W2, dtype=np.float32)
        .astype(bf)
        .reshape(KH, P, O)
        .transpose(1, 0, 2)
        .reshape(P, KH * O)
    )
    if has_b2:
        b2rep = np.broadcast_to(np.asarray(b2, dtype=np.float32).astype(bf), (P, O))
        w2r = np.concatenate([w2r, b2rep], axis=1)
    w2r = np.ascontiguousarray(w2r)
    b1cols = np.asarray(b1, dtype=np.float32).astype(bf).reshape(KH, P).T

    # Pad the global selected-row list to NCORES*CP; padding rows compute
    # garbage that the host scatter ignores.
    rows_pad = np.concatenate(
        [rows, np.full(NCORES * CP - total, rows[-1], dtype=rows.dtype)]
    )

    in_maps = []
    for i in range(NCORES):
        r_i = rows_pad[i * CP : (i + 1) * CP]
        xtT = x_bf[r_i].T.reshape(KD, P, CP)  # [KD, P, CP]
        blocks = []
        for ci, (c0, cw) in enumerate(chunks):
            blocks.append(
                xtT[:, :, c0 : c0 + cw].transpose(1, 0, 2).reshape(P, KD * cw)
            )
            if ci == 0:
                blocks.append(b1cols)
        xt = np.ascontiguousarray(np.concatenate(blocks, axis=1))
        in_maps.append({"xt": xt, "w1r": w1r, "w2r": w2r})
    return nc, in_maps, (rows, total, CP)


def _assemble(res, meta):
    rows, total, CP = meta
    compact = np.concatenate(
        [np.asarray(res.results[i]["out"], dtype=np.float32) for i in range(NCORES)],
        axis=0,
    )[:total]
    out = np.zeros((B, O), dtype=np.float32)
    out[rows] = compact
    return out


def run_full(inputs, trace=False, NWU=8):
    """Shared by kernel() and test harness: returns (out, res)."""
    prep = prepare(**inputs, NWU=NWU)
    if prep is None:
        return np.zeros((B, O), dtype=np.float32), None
    nc, in_maps, meta = prep

    from concourse.bass_utils import run_bass_kernel_spmd

    res = run_bass_kernel_spmd(
        nc, in_maps, core_ids=list(range(NCORES)), trace=trace
    )
    return _assemble(res, meta), res


def kernel(input_data, selection_mask, W1, b1, W2, b2, modality_idx):
    out, _ = run_full(
        dict(
            input_data=input_data,
            selection_mask=selection_mask,
            W1=W1,
            b1=b1,
            W2=W2,
            b2=b2,
            modality_idx=modality_idx,
        )
    )
    return out


# revision 16
# speedup vs baseline: 1.0715x; 1.0715x over previous
"""Trainium2 Bass kernel for nn_AdaptiveModalityEncoder.

Reference computation (per row r of input_data [B, D]):
    sel[r] = selection_mask[r, modality_idx] > 0.5
    out[r] = sel[r] ? gelu(x[r] @ W1 + b1) @ W2 + b2 : 0

Strategy (moe_routing, data-parallel across 8 cores):
  - Host computes the selected-row list, gathers + transposes the selected
    rows (routing metadata/prep), and splits them evenly across the 8
    cores; each core runs a pure dense 2-layer MLP in bf16 (fp32
    accumulate) over its rows and writes a compact batch-major bf16
    output. Host scatters the compact outputs into the zero-filled full
    output.
  - Device kernel is gather/scatter-free: only linear DMAs. Inputs are
    packed host-side into few wide-line DMAs (8-32 KB per partition
    line) because DMA descriptor generation (~9 ns/line, 128 lines per
    SBUF tile) otherwise limits the startup; biases ride along inside
    the X^T and W2 transfers.
  - The Tensor engine p-state ramps over ~5 us of continuous execution
    (427 -> 216 ns per 512-col matmul) and resets on multi-us idle, so
    a short burst of garbage warm-up matmuls covers the engine-boot +
    first-DMA window and real work starts at full speed with zero
    stalls.

Matmul layout: L1 feature-major (H^T = W1^T @ X^T; X^T pre-transposed on
host, W1 repacked so each h-tile's k-slices are contiguous); L2 flips
operands (OUT = (HT)^T @ W2, W2 repacked k-major) so output rows come out
batch-major and stream straight to DRAM.
"""

import sys

sys.path.insert(0, "/opt/trn_rl_repo")

import numpy as np
import ml_dtypes

# Problem constants (hardcoded per harness contract).
B, D, H, O, K = 16384, 1024, 2048, 1024, 4
NCORES = 8
P = 128
KD = D // P  # 8 k-tiles for layer 1
KH = H // P  # 16 k-tiles for layer 2

# W1 h-tile DMA grouping: singles first (feed the first chains ASAP),
# then quads (fewer descriptors per byte).
W1_GROUPS = [[0], [1], [2, 3, 4, 5], [6, 7, 8, 9], [10, 11, 12, 13], [14, 15]]

_GRAPH_CACHE = {}


def _chunks_of(CP):
    out = []
    c0 = 0
    while c0 < CP:
        w = min(512, CP - c0)
        out.append((c0, w))
        c0 += w
    return out


def _build_graph(NG, has_b2, act="gelu", NWU=20):
    """Build + compile the per-core Bass graph. NG = number of 128-row
    tiles per core (CP = NG*128 padded rows). Same graph on all 8 cores."""
    import concourse.mybir as mybir
    import concourse.tile as tile
    from concourse import bacc

    f32 = mybir.dt.float32
    bf16 = mybir.dt.bfloat16
    act_fn = {
        "gelu": mybir.ActivationFunctionType.Gelu_apprx_tanh,
        "tanh": mybir.ActivationFunctionType.Tanh,  # CoreSim stand-in
    }[act]
    copy_fn = mybir.ActivationFunctionType.Copy

    CP = NG * P  # padded rows per core
    chunks = _chunks_of(CP)
    NC = len(chunks)

    nc = bacc.Bacc("TRN2", target_bir_lowering=False, debug=False, num_devices=NCORES)

    # X^T packed per chunk (each chunk's KD k-tiles contiguous), with the
    # b1 bias columns folded in after chunk 0.
    xt_cols = KD * CP + KH
    xt_d = nc.dram_tensor("xt", [P, xt_cols], bf16, kind="ExternalInput")
    w1r_d = nc.dram_tensor("w1r", [P, KH * KD * P], bf16, kind="ExternalInput")
    w2_cols = KH * O + (O if has_b2 else 0)
    w2r_d = nc.dram_tensor("w2r", [P, w2_cols], bf16, kind="ExternalInput")
    out_d = nc.dram_tensor("out", [CP, O], bf16, kind="ExternalOutput")

    with tile.TileContext(nc) as tc:
        with (
            tc.tile_pool(name="w1pool", bufs=len(W1_GROUPS)) as w1pool,
            tc.tile_pool(name="w2pool", bufs=4) as w2pool,
            tc.tile_pool(name="xtp", bufs=NC) as xtp,
            tc.tile_pool(name="htp", bufs=KH) as htp,
            tc.tile_pool(name="outp", bufs=4) as outp,
            tc.tile_pool(name="const", bufs=1) as constp,
            tc.tile_pool(name="ps1", bufs=3, space="PSUM") as ps1,  # layer 1
            tc.tile_pool(name="ps2", bufs=3, space="PSUM") as ps2,  # layer 2
            tc.tile_pool(name="wup", bufs=1, space="PSUM") as wup,
        ):
            # ---- PE warm-up (see module docstring) ----
            wu = constp.tile([P, 512], bf16)
            nc.gpsimd.memset(wu[:], 0.0)
            wuacc = wup.tile([P, 512], f32)
            for i in range(NWU):
                nc.tensor.matmul(
                    wuacc[:], wu[:, 0:P], wu[:], start=True, stop=True
                )

            # ---- DMA issue order = generation order: critical-path first
            # chunk 0 of X^T (+ b1 columns) as ONE wide DMA
            cw0 = chunks[0][1]
            xt0_sb = xtp.tile([P, KD * cw0 + KH], bf16, tag="xt0", name="xt0")
            nc.sync.dma_start(xt0_sb[:], xt_d[:, 0 : KD * cw0 + KH])
            b1_ap = xt0_sb[:, KD * cw0 : KD * cw0 + KH]

            # W1 h-tiles: two singles, then grouped
            w1_tiles = []
            for gi, grp in enumerate(W1_GROUPS):
                t = w1pool.tile(
                    [P, len(grp) * KD * P], bf16, tag="w1", name=f"w1g{gi}"
                )
                w1_tiles.append(t)
                h0 = grp[0]
                nc.sync.dma_start(
                    t[:], w1r_d[:, h0 * KD * P : (h0 + len(grp)) * KD * P]
                )
            # h -> (tile, col offset of that h's KD*P block)
            w1_slice = {}
            for gi, grp in enumerate(W1_GROUPS):
                for j, h in enumerate(grp):
                    w1_slice[h] = (w1_tiles[gi], j * KD * P)

            # remaining X^T chunks
            xt_sb = [None] * NC
            off = KD * cw0 + KH
            for ci in range(1, NC):
                cw = chunks[ci][1]
                t = xtp.tile([P, KD * cw], bf16, tag="xt", name=f"xt{ci}")
                xt_sb[ci] = t
                nc.sync.dma_start(t[:], xt_d[:, off : off + KD * cw])
                off += KD * cw

            # W2 in 4 quads of k-tiles; b2 columns ride in the last quad
            w2_tiles = []
            for qi in range(4):
                extra = O if (has_b2 and qi == 3) else 0
                t = w2pool.tile([P, 4 * O + extra], bf16, tag="w2", name=f"w2q{qi}")
                w2_tiles.append(t)
                nc.sync.dma_start(
                    t[:], w2r_d[:, qi * 4 * O : (qi + 1) * 4 * O + extra]
                )
            b2_ap = w2_tiles[3][:, 4 * O : 5 * O] if has_b2 else None

            def xt_k(ci, k):
                cw = chunks[ci][1]
                t = xt0_sb if ci == 0 else xt_sb[ci]
                return t[:, k * cw : (k + 1) * cw]

            def w2_k(k, lo, hi):
                t = w2_tiles[k // 4]
                base = (k % 4) * O
                return t[:, base + lo : base + hi]

            ht_sb = [
                htp.tile([P, CP], bf16, tag="ht", name=f"htsb{h}")
                for h in range(KH)
            ]

            # ---- compute: per column chunk, L1 then L2 ----
            for ci, (c0, cw) in enumerate(chunks):
                # layer 1: H^T chunk = gelu(W1^T @ X^T + b1)
                for h in range(KH):
                    acc = ps1.tile([P, cw], f32, tag="l1acc", name=f"l1a{ci}_{h}")
                    w1t, w1off = w1_slice[h]
                    for k in range(KD):
                        nc.tensor.matmul(
                            acc[:],
                            w1t[:, w1off + k * P : w1off + (k + 1) * P],
                            xt_k(ci, k),
                            start=(k == 0),
                            stop=(k == KD - 1),
                        )
                    nc.scalar.activation(
                        ht_sb[h][:, c0 : c0 + cw],
                        acc[:],
                        act_fn,
                        bias=b1_ap[:, h : h + 1],
                    )

                # layer 2, batch-major: OUT rows = (HT slice)^T @ W2 + b2
                for rl in range(cw // P):
                    r0 = c0 + rl * P
                    for oc in range(2):
                        acc2 = ps2.tile(
                            [P, 512], f32, tag="l2acc", name=f"l2a{ci}_{rl}_{oc}"
                        )
                        for k in range(KH):
                            nc.tensor.matmul(
                                acc2[:],
                                ht_sb[k][:, r0 : r0 + P],
                                w2_k(k, oc * 512, (oc + 1) * 512),
                                start=(k == 0),
                                stop=(k == KH - 1),
                            )
                        ob = outp.tile(
                            [P, 512], bf16, tag="outsb", name=f"osb{ci}_{rl}_{oc}"
                        )
                        # Evict + write on the scalar engine only: its HWDGE
                        # queue fires ~30 ns after the eviction (same-engine
                        # dependency), where a cross-engine hop costs ~0.7 us.
                        if has_b2:
                            nc.vector.tensor_add(
                                ob[:], acc2[:], b2_ap[:, oc * 512 : (oc + 1) * 512]
                            )
                        else:
                            nc.scalar.activation(ob[:], acc2[:], copy_fn)
                        nc.scalar.dma_start(
                            out_d[r0 : r0 + P, oc * 512 : (oc + 1) * 512], ob[:]
                        )

    nc.compile()
    return nc


def _get_graph(NG, has_b2, act="gelu", NWU=20):
    key = (NG, has_b2, act, NWU)
    if key not in _GRAPH_CACHE:
        _GRAPH_CACHE[key] = _build_graph(NG, has_b2, act, NWU)
    return _GRAPH_CACHE[key]


def prepare(input_data, selection_mask, W1, b1, W2, b2, modality_idx, act="gelu", NWU=20):
    """Host-side routing/sharding prep. Returns (nc, in_maps, meta) or None
    if no rows are selected (output is all zeros)."""
    x = np.asarray(input_data, dtype=np.float32)
    mask = np.asarray(selection_mask, dtype=np.float32)
    midx = int(np.asarray(modality_idx))
    rows = np.nonzero(mask[:, midx] > 0.5)[0]
    total = len(rows)
    if total == 0:
        return None

    T = -(-total // NCORES)  # rows per core
    NG = -(-T // P)
    CP = NG * P
    chunks = _chunks_of(CP)
    has_b2 = bool(np.any(np.asarray(b2)))

    nc = _get_graph(NG, has_b2, act, NWU)

    bf = ml_dtypes.bfloat16
    x_bf = x.astype(bf)
    # W1 repacked: block (h, k) at cols (h*KD + k)*P
    w1r = np.ascontiguousarray(
        np.asarray(W1, dtype=np.float32)
        .astype(bf)
        .reshape(KD, P, KH, P)
        .transpose(1, 2, 0, 3)
        .reshape(P, KH * KD * P)
    )
    # W2 repacked k-major: block k at cols k*O
    w2r = (
        np.asarray(W2, dtype=np.float32)
        .astype(bf)
        .reshape(KH, P, O)
        .transpose(1, 0, 2)
        .reshape(P, KH * O)
    )
    if has_b2:
        b2rep = np.broadcast_to(np.asarray(b2, dtype=np.float32).astype(bf), (P, O))
        w2r = np.concatenate([w2r, b2rep], axis=1)
    w2r = np.ascontiguousarray(w2r)
    b1cols = np.asarray(b1, dtype=np.float32).astype(bf).reshape(KH, P).T

    # Pad the global selected-row list to NCORES*CP; padding rows compute
    # garbage that the host scatter ignores.
    rows_pad = np.concatenate(
        [rows, np.full(NCORES * CP - total, rows[-1], dtype=rows.dtype)]
    )

    in_maps = []
    for i in range(NCORES):
        r_i = rows_pad[i * CP : (i + 1) * CP]
        xtT = x_bf[r_i].T.reshape(KD, P, CP)  # [KD, P, CP]
        blocks = []
        for ci, (c0, cw) in enumerate(chunks):
            blocks.append(
                xtT[:, :, c0 : c0 + cw].transpose(1, 0, 2).reshape(P, KD * cw)
            )
            if ci == 0:
                blocks.append(b1cols)
        xt = np.ascontiguousarray(np.concatenate(blocks, axis=1))
        in_maps.append({"xt": xt, "w1r": w1r, "w2r": w2r})
    return nc, in_maps, (rows, total, CP)


def _assemble(res, meta):
    rows, total, CP = meta
    compact = np.concatenate(
        [np.asarray(res.results[i]["out"], dtype=np.float32) for i in range(NCORES)],
        axis=0,
    )[:total]
    out = np.zeros((B, O), dtype=np.float32)
    out[rows] = compact
    return out


def run_full(inputs, trace=False, NWU=20):
    """Shared by kernel() and test harness: returns (out, res)."""
    prep = prepare(**inputs, NWU=NWU)
    if prep is None:
        return np.zeros((B, O), dtype=np.float32), None
    nc, in_maps, meta = prep

    from concourse.bass_utils import run_bass_kernel_spmd

    res = run_bass_kernel_spmd(
        nc, in_maps, core_ids=list(range(NCORES)), trace=trace
    )
    return _assemble(res, meta), res


def kernel(input_data, selection_mask, W1, b1, W2, b2, modality_idx):
    out, _ = run_full(
        dict(
            input_data=input_data,
            selection_mask=selection_mask,
            W1=W1,
            b1=b1,
            W2=W2,
            b2=b2,
            modality_idx=modality_idx,
        )
    )
    return out
